# revision 22
# baseline (speedup 1.0000x reference)
"""Trainium2 Bass kernel for nn_Chambers: 6 per-chamber MLPs over a shared
reservoir input, followed by 5 coupled-chamber fixed-point iterations.

Data-parallel over 8 NeuronCores: each core processes B/8 = 32768 rows.

v2 design (ACT-engine-bound; see work/ notes):
  - MLP is feature-major with [128,2048] PSUM tiles (2-tile rotation = all
    8 banks) so every silu ACT op is as large as PSUM allows:
    L1 = 6 ops/chunk (per-chamber, per-partition bias), L2 = 3 ops/chunk
    (pair-packed), L3 = 2 ops/chunk (c0-3 full tile + c4/c5 stacked across
    chunk halves so no ACT lanes are wasted).
  - L4 per-chamber dot products are K-stacked accumulating matmuls into a
    [6,2048] PSUM strip that is DMA-scattered directly into the batch-major
    rawbm tile (no [6,*] DVE copies anywhere).
  - Coupling runs batch-major [128, 6F] with sin|cos concatenated in one
    [128,12F] bf16 tile; the 15 symmetric C-pair AXPYs update P and Q
    together through 4D strided views (bf16 -> DVE 2x), decay is one
    pattern-tile multiply, and sigmoid is one merged tanh (b4 folded via a
    pattern add). State stays f32.
  - Outputs are written per chamber as contiguous [128,F] DMAs into
    chamber-major [6,R] HBM tensors; host transposes (row = r0 + p*F + f
    is exactly the slice row order).
"""

import numpy as np

# ---- problem constants (fixed by the task; kernel.py must be self-contained)
B = 262144
RES_DIM = 100
NCH = 6
CF_ITERS = 5
CF_K = 0.02
DECAY = np.array([0.9, 0.93, 0.85, 0.97, 0.88, 0.94], dtype=np.float32)
COUPLING = np.array([
    [0.0, -0.3, 0.6, 0.4, -0.2, 0.3],
    [-0.3, 0.0, -0.5, -0.7, 0.6, 0.4],
    [0.6, -0.5, 0.0, 0.3, -0.3, 0.2],
    [0.4, -0.7, 0.3, 0.0, -0.4, 0.5],
    [-0.2, 0.6, -0.3, -0.4, 0.0, 0.3],
    [0.3, 0.4, 0.2, 0.5, 0.3, 0.0]], dtype=np.float32)
N_CORES = 8
R_CORE = B // N_CORES          # 32768 rows per core
CHUNK = 2048                   # rows per MLP chunk
HALF_PI = float(np.pi / 2.0)
DEFAULT_SLICES = (8192, 8192, 8192, 4096, 4096)

_BUILD_CACHE = {}


def _build(R, slice_sizes):
    """Emit + compile the per-core SPMD program."""
    from contextlib import ExitStack
    import concourse.bass as bass
    import concourse.mybir as mybir
    from concourse import bacc, tile

    f32 = mybir.dt.float32
    bf16 = mybir.dt.bfloat16
    AF = mybir.ActivationFunctionType
    OP = mybir.AluOpType

    assert sum(slice_sizes) == R and all(s % CHUNK == 0 for s in slice_sizes)
    KC = (CF_K * COUPLING).astype(np.float64)
    init_pairs = [(0, 1), (2, 3), (4, 5)]
    rest_pairs = [(i, j) for i in range(6) for j in range(i + 1, 6)
                  if (i, j) not in init_pairs]
    fs = sorted({s // 128 for s in slice_sizes}, reverse=True)  # distinct F values
    f_off = {}
    off = 0
    for F in fs:
        f_off[F] = off
        off += 6 * F
    PATW = off                                   # pattern tensor width

    nc = bacc.Bacc("TRN2", target_bir_lowering=False, debug=False,
                   num_devices=N_CORES)
    res = nc.dram_tensor("res_t", [RES_DIM, R], bf16, kind="ExternalInput").ap()
    w1t = nc.dram_tensor("w1t", [RES_DIM, 6 * 128], bf16, kind="ExternalInput").ap()
    b1t = nc.dram_tensor("b1t", [128, 6], f32, kind="ExternalInput").ap()
    w2t = nc.dram_tensor("w2t", [128, 6 * 64], bf16, kind="ExternalInput").ap()
    b2p = nc.dram_tensor("b2p", [128, 3], f32, kind="ExternalInput").ap()
    # w3t holds W3^T twice (rows 0-63 and 64-127) for row-tiled matmuls
    w3t = nc.dram_tensor("w3t", [128, 6 * 32], bf16, kind="ExternalInput").ap()
    b3x = nc.dram_tensor("b3x", [128, 1], f32, kind="ExternalInput").ap()
    b3y = nc.dram_tensor("b3y", [128, 1], f32, kind="ExternalInput").ap()
    w4a = nc.dram_tensor("w4a", [128, 6], bf16, kind="ExternalInput").ap()
    w4b = nc.dram_tensor("w4b", [128, 6], bf16, kind="ExternalInput").ap()
    cst = nc.dram_tensor("cst", [128, 1], f32, kind="ExternalInput").ap()
    decp = nc.dram_tensor("decp", [128, PATW], f32, kind="ExternalInput").ap()
    b4p = nc.dram_tensor("b4p", [128, PATW], f32, kind="ExternalInput").ap()
    act_o = nc.dram_tensor("act_o", [6, R], f32, kind="ExternalOutput").ap()
    raw_o = nc.dram_tensor("raw_o", [6, R], f32, kind="ExternalOutput").ap()

    def emit():
        with tile.TileContext(nc) as tc, ExitStack() as ctx:
            wp = ctx.enter_context(tc.tile_pool(name="w", bufs=1))
            # critical-path weights (L1 needs these first) on the sync
            # HWDGE queue; the rest on gpsimd; coupling patterns (first
            # used ~100us in) last
            t_w1t = wp.tile([RES_DIM, 6 * 128], bf16, tag="w1t")
            nc.sync.dma_start(t_w1t[:], w1t)
            t_b1t = wp.tile([128, 6], f32, tag="b1t")
            nc.sync.dma_start(t_b1t[:], b1t)
            t_w2t = wp.tile([128, 6 * 64], bf16, tag="w2t")
            nc.gpsimd.dma_start(t_w2t[:], w2t)
            t_b2p = wp.tile([128, 3], f32, tag="b2p")
            nc.gpsimd.dma_start(t_b2p[:], b2p)
            t_w3t = wp.tile([128, 6 * 32], bf16, tag="w3t")
            nc.gpsimd.dma_start(t_w3t[:], w3t)
            t_b3x = wp.tile([128, 1], f32, tag="b3x")
            nc.gpsimd.dma_start(t_b3x[:], b3x)
            t_b3y = wp.tile([128, 1], f32, tag="b3y")
            nc.gpsimd.dma_start(t_b3y[:], b3y)
            t_w4a = wp.tile([128, 6], bf16, tag="w4a")
            nc.gpsimd.dma_start(t_w4a[:], w4a)
            t_w4b = wp.tile([128, 6], bf16, tag="w4b")
            nc.gpsimd.dma_start(t_w4b[:], w4b)
            t_cst = wp.tile([128, 1], f32, tag="cst")
            nc.gpsimd.dma_start(t_cst[:], cst)
            t_decp = wp.tile([128, PATW], f32, tag="decp")
            nc.gpsimd.dma_start(t_decp[:], decp)
            t_b4p = wp.tile([128, PATW], f32, tag="b4p")
            nc.gpsimd.dma_start(t_b4p[:], b4p)

            p_rT = ctx.enter_context(tc.tile_pool(name="rT", bufs=3))
            p_mm = ctx.enter_context(tc.tile_pool(name="pmm", bufs=4, space="PSUM"))
            p_h1 = ctx.enter_context(tc.tile_pool(name="h1", bufs=2))
            p_h2 = ctx.enter_context(tc.tile_pool(name="h2", bufs=2))
            p_h3 = ctx.enter_context(tc.tile_pool(name="h3", bufs=2))
            p_rsb = ctx.enter_context(tc.tile_pool(name="rsb", bufs=2))
            p_bm = ctx.enter_context(tc.tile_pool(name="bm", bufs=2))
            p_cpl = ctx.enter_context(tc.tile_pool(name="cpl", bufs=2))

            # 3D pair views on one u-half of a [128, 12F] tile (u=0 first
            # 6F columns, u=1 second): verifier caps TS/STT APs at 3 dims.
            def pq_out(t_ap, u, i, j, F):
                x = t_ap[:, u * 6 * F:(u + 1) * 6 * F].rearrange(
                    "p (c f) -> p c f", f=F)
                return x[:, i:j + 1:(j - i), :]

            def pq_src(t_ap, u, i, j, F):
                # c-blocks [j, i] (swapped) via negative step
                x = t_ap[:, u * 6 * F:(u + 1) * 6 * F].rearrange(
                    "p (c f) -> p c f", f=F)
                if i == 0:
                    return x[:, j::-(j - i), :][:, 0:2, :]
                return x[:, j:i - 1:-(j - i), :]

            def swap_u(t_ap, F):
                x = t_ap.rearrange("p (u x) -> p u x", u=2)
                return x[:, 1::-1, :]

            FMAX = max(slice_sizes) // 128
            H = CHUNK // 2

            def ctile(tag, w, dt, F):
                t = p_cpl.tile([128, w * FMAX], dt, tag=tag)
                return t[:, 0:w * F]

            def emit_front(g, rT, h1, h2):
                # L1 (+ L2 when a pair completes); [128,1024] PSUM tiles
                for c in range(6):
                    for hh in range(2):
                        ps = p_mm.tile([128, H], f32, tag="mm")
                        for q in range(H // 512):
                            o = hh * H + q * 512
                            nc.tensor.matmul(
                                ps[:, q * 512:(q + 1) * 512],
                                t_w1t[:, c * 128:(c + 1) * 128],
                                rT[:, o:o + 512])
                        nc.scalar.activation(
                            h1[:, c * CHUNK + hh * H:c * CHUNK + (hh + 1) * H],
                            ps[:], AF.Silu, bias=t_b1t[:, c:c + 1])
                    if c % 2 == 1:
                        p = c // 2
                        for hh in range(2):
                            ps2 = p_mm.tile([128, H], f32, tag="mm")
                            for q in range(H // 512):
                                o = hh * H + q * 512
                                nc.tensor.matmul(
                                    ps2[0:64, q * 512:(q + 1) * 512],
                                    t_w2t[:, (2 * p) * 64:(2 * p + 1) * 64],
                                    h1[:, 2 * p * CHUNK + o:
                                       2 * p * CHUNK + o + 512],
                                    tile_position=(0, 0))
                                nc.tensor.matmul(
                                    ps2[64:128, q * 512:(q + 1) * 512],
                                    t_w2t[:, (2 * p + 1) * 64:(2 * p + 2) * 64],
                                    h1[:, (2 * p + 1) * CHUNK + o:
                                       (2 * p + 1) * CHUNK + o + 512],
                                    tile_position=(0, 64))
                            nc.scalar.activation(
                                h2[:, p * CHUNK + hh * H:p * CHUNK + (hh + 1) * H],
                                ps2[:], AF.Silu, bias=t_b2p[:, p:p + 1])

            def emit_tail(st):
                # L3 + L4 + scatter for a chunk (deferred one chunk so the
                # fill-heavy stage interleaves with the next chunk's L1/L2)
                h2, rawbm, k, F, PPC = st
                h3 = p_h3.tile([128, 3 * CHUNK // 2], bf16, tag="h3")
                for hh in range(2):
                    psx = p_mm.tile([128, H], f32, tag="mm")
                    for c in range(4):
                        p = c // 2
                        half = c % 2          # which 64-row half of the pair tile
                        for q in range(H // 512):
                            o = hh * H + q * 512
                            nc.tensor.matmul(
                                psx[32 * c:32 * (c + 1), q * 512:(q + 1) * 512],
                                t_w3t[64 * half:64 * half + 64,
                                      c * 32:(c + 1) * 32],
                                h2[64 * half:64 * half + 64,
                                   p * CHUNK + o:p * CHUNK + o + 512],
                                tile_position=(64 * half, 32 * c))
                    # (4 chambers run concurrently per q: distinct
                    # (row,col) groups (0,0),(64,32),(0,64),(64,96))
                    nc.scalar.activation(h3[:, hh * H:(hh + 1) * H], psx[:],
                                         AF.Silu, bias=t_b3x[:])

                # L3 Y: c4/c5, chunk halves stacked in partitions:
                # [0:32]=c4 cols 0:H, [32:64]=c5 cols 0:H,
                # [64:96]=c4 cols H:2H, [96:128]=c5 cols H:2H
                psy = p_mm.tile([128, H], f32, tag="mm")
                for hh in range(2):                 # chunk half
                    for ci, c in enumerate((4, 5)):
                        half = ci                   # c4 lower, c5 upper h2 rows
                        pb = 64 * hh + 32 * ci
                        for q in range(H // 512):
                            o = hh * H + q * 512
                            nc.tensor.matmul(
                                psy[pb:pb + 32, q * 512:(q + 1) * 512],
                                t_w3t[64 * half:64 * half + 64,
                                      c * 32:(c + 1) * 32],
                                h2[64 * half:64 * half + 64,
                                   2 * CHUNK + o:2 * CHUNK + o + 512],
                                tile_position=(64 * half, pb))
                nc.scalar.activation(h3[:, CHUNK:CHUNK + H], psy[:],
                                     AF.Silu, bias=t_b3y[:])

                # L4: raw[0:6] accumulating K-stacked matmuls
                rsb = p_rsb.tile([6, CHUNK], f32, tag="rsb")
                for hh in range(2):
                    ps4 = p_mm.tile([128, H], f32, tag="mm")
                    for q in range(H // 512):
                        o = hh * H + q * 512
                        nc.tensor.matmul(
                            ps4[0:6, q * 512:(q + 1) * 512], t_w4a[:, 0:6],
                            h3[:, o:o + 512],
                            start=True, stop=False, tile_position=(0, 0))
                        # c4/c5 features: partitions 64*hh:64*hh+64 of h3y
                        nc.tensor.matmul(
                            ps4[0:6, q * 512:(q + 1) * 512],
                            t_w4b[64 * hh:64 * hh + 64, 0:6],
                            h3[64 * hh:64 * hh + 64,
                               CHUNK + q * 512:CHUNK + (q + 1) * 512],
                            start=False, stop=True,
                            tile_position=(64 * hh, 0))
                    nc.vector.tensor_copy(rsb[:, hh * H:(hh + 1) * H],
                                          ps4[0:6, :])
                # scatter into batch-major rawbm
                for c in range(6):
                    nc.gpsimd.dma_start(
                        rawbm[k * PPC:(k + 1) * PPC, c * F:(c + 1) * F],
                        rsb[c:c + 1, :].rearrange("o (a f) -> o a f", f=F))

            def emit_post(sp):
                # raw outputs + coupling + act outputs for a finished slice
                rawbm, F, po, r0, srows = sp
                for c in range(6):
                    q = nc.sync if c % 2 == 0 else nc.gpsimd
                    q.dma_start(
                        raw_o[c:c + 1, r0:r0 + srows]
                        .rearrange("o (p f) -> (o p) f", f=F),
                        rawbm[:, c * F:(c + 1) * F])

                rawb = ctile("RB", 6, f32, F)
                nc.vector.tensor_tensor(rawb[:], rawbm[:, 0:6 * F],
                                        t_b4p[:, po:po + 6 * F], OP.add)
                A = ctile("A", 6, f32, F)
                tt6 = ctile("T6", 6, bf16, F)
                nc.scalar.activation(tt6[:], rawb[:], AF.Tanh, scale=0.5)
                nc.vector.tensor_scalar(A[:], tt6[:], 0.5, 0.5, OP.mult, OP.add)
                for it in range(CF_ITERS):
                    # D2 = [decay*A | decay*A + pi/2]; one sin ACT covers
                    # sin and cos halves
                    D2 = ctile("D2", 12, f32, F)
                    D = D2[:, 0:6 * F]
                    nc.vector.tensor_tensor(D, A[:],
                                            t_decp[:, po:po + 6 * F], OP.mult)
                    nc.vector.tensor_scalar(D2[:, 6 * F:12 * F], D,
                                            HALF_PI, None, OP.add)
                    SC = ctile("SC", 12, bf16, F)
                    nc.scalar.activation(SC[:], D2[:], AF.Sin)
                    PQ = ctile("PQ", 12, bf16, F)
                    for u in range(2):
                        for (i, j) in init_pairs:
                            nc.vector.tensor_scalar(pq_out(PQ, u, i, j, F),
                                                    pq_src(SC, u, i, j, F),
                                                    float(KC[i][j]), None,
                                                    OP.mult)
                        for (i, j) in rest_pairs:
                            nc.vector.scalar_tensor_tensor(
                                pq_out(PQ, u, i, j, F), pq_src(SC, u, i, j, F),
                                float(KC[i][j]), pq_out(PQ, u, i, j, F),
                                OP.mult, OP.add)
                    U = ctile("U", 12, bf16, F)
                    nc.vector.tensor_tensor(
                        U.rearrange("p (u x) -> p u x", u=2),
                        PQ.rearrange("p (u x) -> p u x", u=2),
                        swap_u(SC, F), OP.mult)
                    DD = ctile("DD", 6, bf16, F)
                    nc.vector.tensor_tensor(DD[:], U[:, 0:6 * F],
                                            U[:, 6 * F:12 * F], OP.subtract)
                    V = ctile("V", 6, f32, F)
                    nc.vector.tensor_tensor(V[:], D, DD[:], OP.add)
                    A = ctile("A", 6, f32, F)
                    nc.vector.tensor_scalar(A[:], V[:], 0.0, 1.0, OP.max, OP.min)

                for c in range(6):
                    q = nc.sync if c % 2 == 0 else nc.gpsimd
                    q.dma_start(
                        act_o[c:c + 1, r0:r0 + srows]
                        .rearrange("o (p f) -> (o p) f", f=F),
                        A[:, c * F:(c + 1) * F])

            pend_tail = None        # (tail_state, post_state_or_None)
            g0 = 0
            for s, srows in enumerate(slice_sizes):
                F = srows // 128
                PPC = CHUNK // F
                po = f_off[F]
                r0 = g0 * CHUNK
                cps = srows // CHUNK
                rawbm = p_bm.tile([128, 6 * FMAX], f32, tag="rawbm")
                for k in range(cps):
                    g = g0 + k                         # global chunk id
                    rT = p_rT.tile([RES_DIM, CHUNK], bf16, tag="rT")
                    nc.sync.dma_start(rT[:], res[:, g * CHUNK:(g + 1) * CHUNK])
                    h1 = p_h1.tile([128, 6 * CHUNK], bf16, tag="h1")
                    h2 = p_h2.tile([128, 3 * CHUNK], bf16, tag="h2")
                    emit_front(g, rT, h1, h2)
                    if pend_tail is not None:
                        st, post = pend_tail
                        emit_tail(st)
                        if post is not None:
                            emit_post(post)
                    post = ((rawbm, F, po, r0, srows)
                            if k == cps - 1 else None)
                    pend_tail = ((h2, rawbm, k, F, PPC), post)
                g0 += cps
            st, post = pend_tail
            emit_tail(st)
            emit_post(post)
    return nc, emit


def prep_weights(W1, b1, W2, b2, W3, b3, W4, b4, slice_sizes):
    """Host-side weight layout preparation."""
    import ml_dtypes
    bf16 = ml_dtypes.bfloat16
    d = {}
    d["w1t"] = np.ascontiguousarray(
        W1.transpose(2, 0, 1).reshape(RES_DIM, 6 * 128)).astype(bf16)
    d["b1t"] = np.ascontiguousarray(b1.T)                      # [128, 6]
    d["w2t"] = np.ascontiguousarray(
        W2.transpose(2, 0, 1).reshape(128, 6 * 64)).astype(bf16)
    b2p = np.zeros((128, 3), np.float32)
    for p in range(3):
        b2p[0:64, p] = b2[2 * p]
        b2p[64:128, p] = b2[2 * p + 1]
    d["b2p"] = b2p
    w3t_h = W3.transpose(2, 0, 1).reshape(64, 6 * 32)
    d["w3t"] = np.ascontiguousarray(
        np.concatenate([w3t_h, w3t_h], axis=0)).astype(bf16)
    b3x = np.zeros((128, 1), np.float32)
    for c in range(4):
        b3x[32 * c:32 * (c + 1), 0] = b3[c]
    d["b3x"] = b3x
    b3y = np.zeros((128, 1), np.float32)
    b3y[0:32, 0] = b3[4]
    b3y[32:64, 0] = b3[5]
    b3y[64:96, 0] = b3[4]
    b3y[96:128, 0] = b3[5]
    d["b3y"] = b3y
    w4a = np.zeros((128, 6), np.float32)
    for c in range(4):
        w4a[32 * c:32 * (c + 1), c] = W4[c, 0, :]
    d["w4a"] = w4a.astype(bf16)
    w4b = np.zeros((128, 6), np.float32)
    w4b[0:32, 4] = W4[4, 0, :]
    w4b[32:64, 5] = W4[5, 0, :]
    w4b[64:96, 4] = W4[4, 0, :]
    w4b[96:128, 5] = W4[5, 0, :]
    d["w4b"] = w4b.astype(bf16)
    d["cst"] = np.full((128, 1), HALF_PI, np.float32)
    fs = sorted({s // 128 for s in slice_sizes}, reverse=True)
    patw = sum(6 * F for F in fs)
    decp = np.zeros((128, patw), np.float32)
    b4p = np.zeros((128, patw), np.float32)
    off = 0
    for F in fs:
        for c in range(6):
            decp[:, off + c * F:off + (c + 1) * F] = DECAY[c]
            b4p[:, off + c * F:off + (c + 1) * F] = b4[c, 0]
        off += 6 * F
    d["decp"] = decp
    d["b4p"] = b4p
    d["_b4"] = np.ascontiguousarray(b4[:, 0])                  # host-only
    return d


def build_program(R=R_CORE, slice_sizes=DEFAULT_SLICES):
    """Build + bacc-compile the program (cached)."""
    key = (R, tuple(slice_sizes))
    if key in _BUILD_CACHE:
        return _BUILD_CACHE[key]
    nc, emit = _build(R, list(slice_sizes))
    emit()
    nc.compile()
    _BUILD_CACHE[key] = nc
    return nc


def kernel(res, W1, b1, W2, b2, W3, b3, W4, b4, coupling):
    """Full-input entry point: shards res over 8 cores, runs the SPMD
    kernel, gathers and returns (act, raw) like the reference."""
    from concourse.bass_utils import run_bass_kernel_spmd

    res = np.ascontiguousarray(np.asarray(res, np.float32))
    W1 = np.asarray(W1, np.float32); b1 = np.asarray(b1, np.float32)
    W2 = np.asarray(W2, np.float32); b2 = np.asarray(b2, np.float32)
    W3 = np.asarray(W3, np.float32); b3 = np.asarray(b3, np.float32)
    W4 = np.asarray(W4, np.float32); b4 = np.asarray(b4, np.float32)

    wd = prep_weights(W1, b1, W2, b2, W3, b3, W4, b4, DEFAULT_SLICES)
    b4vec = wd.pop("_b4")
    nc = build_program(R_CORE)

    import ml_dtypes
    res_t = np.ascontiguousarray(res.T.astype(ml_dtypes.bfloat16))  # [100, B]
    in_maps = []
    for i in range(N_CORES):
        m = dict(wd)
        m["res_t"] = np.ascontiguousarray(res_t[:, i * R_CORE:(i + 1) * R_CORE])
        in_maps.append(m)
    out = run_bass_kernel_spmd(nc, in_maps, list(range(N_CORES)))
    act = np.concatenate(
        [np.ascontiguousarray(out.results[i]["act_o"].T) for i in range(N_CORES)],
        axis=0)
    raw = np.concatenate(
        [np.ascontiguousarray(out.results[i]["raw_o"].T) for i in range(N_CORES)],
        axis=0)
    raw = raw + b4vec[None, :]
    return act.astype(np.float32), raw.astype(np.float32)


# revision 23
# speedup vs baseline: 1.0201x; 1.0201x over previous
"""Trainium2 Bass kernel for nn_Chambers: 6 per-chamber MLPs over a shared
reservoir input, followed by 5 coupled-chamber fixed-point iterations.

Data-parallel over 8 NeuronCores: each core processes B/8 = 32768 rows.

v2 design (ACT-engine-bound; see work/ notes):
  - MLP is feature-major with [128,2048] PSUM tiles (2-tile rotation = all
    8 banks) so every silu ACT op is as large as PSUM allows:
    L1 = 6 ops/chunk (per-chamber, per-partition bias), L2 = 3 ops/chunk
    (pair-packed), L3 = 2 ops/chunk (c0-3 full tile + c4/c5 stacked across
    chunk halves so no ACT lanes are wasted).
  - L4 per-chamber dot products are K-stacked accumulating matmuls into a
    [6,2048] PSUM strip that is DMA-scattered directly into the batch-major
    rawbm tile (no [6,*] DVE copies anywhere).
  - Coupling runs batch-major [128, 6F] with sin|cos concatenated in one
    [128,12F] bf16 tile; the 15 symmetric C-pair AXPYs update P and Q
    together through 4D strided views (bf16 -> DVE 2x), decay is one
    pattern-tile multiply, and sigmoid is one merged tanh (b4 folded via a
    pattern add). State stays f32.
  - Outputs are written per chamber as contiguous [128,F] DMAs into
    chamber-major [6,R] HBM tensors; host transposes (row = r0 + p*F + f
    is exactly the slice row order).
"""

import numpy as np

# ---- problem constants (fixed by the task; kernel.py must be self-contained)
B = 262144
RES_DIM = 100
NCH = 6
CF_ITERS = 5
CF_K = 0.02
DECAY = np.array([0.9, 0.93, 0.85, 0.97, 0.88, 0.94], dtype=np.float32)
COUPLING = np.array([
    [0.0, -0.3, 0.6, 0.4, -0.2, 0.3],
    [-0.3, 0.0, -0.5, -0.7, 0.6, 0.4],
    [0.6, -0.5, 0.0, 0.3, -0.3, 0.2],
    [0.4, -0.7, 0.3, 0.0, -0.4, 0.5],
    [-0.2, 0.6, -0.3, -0.4, 0.0, 0.3],
    [0.3, 0.4, 0.2, 0.5, 0.3, 0.0]], dtype=np.float32)
N_CORES = 8
R_CORE = B // N_CORES          # 32768 rows per core
CHUNK = 2048                   # rows per MLP chunk
HALF_PI = float(np.pi / 2.0)
DEFAULT_SLICES = (8192, 8192, 8192, 4096, 4096)

_BUILD_CACHE = {}


def _build(R, slice_sizes):
    """Emit + compile the per-core SPMD program."""
    from contextlib import ExitStack
    import concourse.bass as bass
    import concourse.mybir as mybir
    from concourse import bacc, tile

    f32 = mybir.dt.float32
    bf16 = mybir.dt.bfloat16
    AF = mybir.ActivationFunctionType
    OP = mybir.AluOpType

    assert sum(slice_sizes) == R and all(s % CHUNK == 0 for s in slice_sizes)
    KC = (CF_K * COUPLING).astype(np.float64)
    init_pairs = [(0, 1), (2, 3), (4, 5)]
    rest_pairs = [(i, j) for i in range(6) for j in range(i + 1, 6)
                  if (i, j) not in init_pairs]
    fs = sorted({s // 128 for s in slice_sizes}, reverse=True)  # distinct F values
    f_off = {}
    off = 0
    for F in fs:
        f_off[F] = off
        off += 6 * F
    PATW = off                                   # pattern tensor width

    nc = bacc.Bacc("TRN2", target_bir_lowering=False, debug=False,
                   num_devices=N_CORES)
    res = nc.dram_tensor("res_t", [RES_DIM, R], bf16, kind="ExternalInput").ap()
    w1t = nc.dram_tensor("w1t", [RES_DIM, 6 * 128], bf16, kind="ExternalInput").ap()
    b1t = nc.dram_tensor("b1t", [128, 6], f32, kind="ExternalInput").ap()
    w2t = nc.dram_tensor("w2t", [128, 6 * 64], bf16, kind="ExternalInput").ap()
    b2p = nc.dram_tensor("b2p", [128, 3], f32, kind="ExternalInput").ap()
    # w3t holds W3^T twice (rows 0-63 and 64-127) for row-tiled matmuls
    w3t = nc.dram_tensor("w3t", [128, 6 * 32], bf16, kind="ExternalInput").ap()
    b3x = nc.dram_tensor("b3x", [128, 1], f32, kind="ExternalInput").ap()
    b3y = nc.dram_tensor("b3y", [128, 1], f32, kind="ExternalInput").ap()
    w4a = nc.dram_tensor("w4a", [128, 6], bf16, kind="ExternalInput").ap()
    w4b = nc.dram_tensor("w4b", [128, 6], bf16, kind="ExternalInput").ap()
    cst = nc.dram_tensor("cst", [128, 1], f32, kind="ExternalInput").ap()
    decp = nc.dram_tensor("decp", [128, PATW], f32, kind="ExternalInput").ap()
    b4p = nc.dram_tensor("b4p", [128, PATW], f32, kind="ExternalInput").ap()
    act_o = nc.dram_tensor("act_o", [6, R], f32, kind="ExternalOutput").ap()
    raw_o = nc.dram_tensor("raw_o", [6, R], f32, kind="ExternalOutput").ap()

    def emit():
        with tile.TileContext(nc) as tc, ExitStack() as ctx:
            wp = ctx.enter_context(tc.tile_pool(name="w", bufs=1))
            # critical-path weights (L1 needs these first) on the sync
            # HWDGE queue; the rest on gpsimd; coupling patterns (first
            # used ~100us in) last
            t_w1t = wp.tile([RES_DIM, 6 * 128], bf16, tag="w1t")
            nc.sync.dma_start(t_w1t[:], w1t)
            t_b1t = wp.tile([128, 6], f32, tag="b1t")
            nc.sync.dma_start(t_b1t[:], b1t)
            t_w2t = wp.tile([128, 6 * 64], bf16, tag="w2t")
            nc.gpsimd.dma_start(t_w2t[:], w2t)
            t_b2p = wp.tile([128, 3], f32, tag="b2p")
            nc.gpsimd.dma_start(t_b2p[:], b2p)
            t_w3t = wp.tile([128, 6 * 32], bf16, tag="w3t")
            nc.gpsimd.dma_start(t_w3t[:], w3t)
            t_b3x = wp.tile([128, 1], f32, tag="b3x")
            nc.gpsimd.dma_start(t_b3x[:], b3x)
            t_b3y = wp.tile([128, 1], f32, tag="b3y")
            nc.gpsimd.dma_start(t_b3y[:], b3y)
            t_w4a = wp.tile([128, 6], bf16, tag="w4a")
            nc.gpsimd.dma_start(t_w4a[:], w4a)
            t_w4b = wp.tile([128, 6], bf16, tag="w4b")
            nc.gpsimd.dma_start(t_w4b[:], w4b)
            t_cst = wp.tile([128, 1], f32, tag="cst")
            nc.gpsimd.dma_start(t_cst[:], cst)
            t_decp = wp.tile([128, PATW], f32, tag="decp")
            nc.gpsimd.dma_start(t_decp[:], decp)
            t_b4p = wp.tile([128, PATW], f32, tag="b4p")
            nc.gpsimd.dma_start(t_b4p[:], b4p)

            p_rT = ctx.enter_context(tc.tile_pool(name="rT", bufs=3))
            p_mm = ctx.enter_context(tc.tile_pool(name="pmm", bufs=4, space="PSUM"))
            p_h1 = ctx.enter_context(tc.tile_pool(name="h1", bufs=2))
            p_h2 = ctx.enter_context(tc.tile_pool(name="h2", bufs=2))
            p_h3 = ctx.enter_context(tc.tile_pool(name="h3", bufs=2))
            p_rsb = ctx.enter_context(tc.tile_pool(name="rsb", bufs=2))
            p_bm = ctx.enter_context(tc.tile_pool(name="bm", bufs=2))
            p_cpl = ctx.enter_context(tc.tile_pool(name="cpl", bufs=2))

            # 3D pair views on one u-half of a [128, 12F] tile (u=0 first
            # 6F columns, u=1 second): verifier caps TS/STT APs at 3 dims.
            def pq_out(t_ap, u, i, j, F):
                x = t_ap[:, u * 6 * F:(u + 1) * 6 * F].rearrange(
                    "p (c f) -> p c f", f=F)
                return x[:, i:j + 1:(j - i), :]

            def pq_src(t_ap, u, i, j, F):
                # c-blocks [j, i] (swapped) via negative step
                x = t_ap[:, u * 6 * F:(u + 1) * 6 * F].rearrange(
                    "p (c f) -> p c f", f=F)
                if i == 0:
                    return x[:, j::-(j - i), :][:, 0:2, :]
                return x[:, j:i - 1:-(j - i), :]

            def swap_u(t_ap, F):
                x = t_ap.rearrange("p (u x) -> p u x", u=2)
                return x[:, 1::-1, :]

            FMAX = max(slice_sizes) // 128
            g0 = 0
            for s, srows in enumerate(slice_sizes):
                F = srows // 128
                PPC = CHUNK // F
                po = f_off[F]
                chunks_per_slice = srows // CHUNK
                rawbm = p_bm.tile([128, 6 * FMAX], f32, tag="rawbm")
                for k in range(chunks_per_slice):
                    g = g0 + k                         # global chunk id
                    rT = p_rT.tile([RES_DIM, CHUNK], bf16, tag="rT")
                    nc.sync.dma_start(rT[:], res[:, g * CHUNK:(g + 1) * CHUNK])

                    # -- L1 (+ L2 when a pair completes); [128,1024] PSUM
                    # tiles, 4-deep rotation
                    h1 = p_h1.tile([128, 6 * CHUNK], bf16, tag="h1")
                    h2 = p_h2.tile([128, 3 * CHUNK], bf16, tag="h2")
                    H = CHUNK // 2
                    for c in range(6):
                        for hh in range(2):
                            ps = p_mm.tile([128, H], f32, tag="mm")
                            for q in range(H // 512):
                                o = hh * H + q * 512
                                nc.tensor.matmul(
                                    ps[:, q * 512:(q + 1) * 512],
                                    t_w1t[:, c * 128:(c + 1) * 128],
                                    rT[:, o:o + 512])
                            nc.scalar.activation(
                                h1[:, c * CHUNK + hh * H:c * CHUNK + (hh + 1) * H],
                                ps[:], AF.Silu, bias=t_b1t[:, c:c + 1])
                        if c % 2 == 1:
                            p = c // 2
                            for hh in range(2):
                                ps2 = p_mm.tile([128, H], f32, tag="mm")
                                for q in range(H // 512):
                                    o = hh * H + q * 512
                                    nc.tensor.matmul(
                                        ps2[0:64, q * 512:(q + 1) * 512],
                                        t_w2t[:, (2 * p) * 64:(2 * p + 1) * 64],
                                        h1[:, 2 * p * CHUNK + o:
                                           2 * p * CHUNK + o + 512],
                                        tile_position=(0, 0))
                                    nc.tensor.matmul(
                                        ps2[64:128, q * 512:(q + 1) * 512],
                                        t_w2t[:, (2 * p + 1) * 64:(2 * p + 2) * 64],
                                        h1[:, (2 * p + 1) * CHUNK + o:
                                           (2 * p + 1) * CHUNK + o + 512],
                                        tile_position=(0, 64))
                                nc.scalar.activation(
                                    h2[:, p * CHUNK + hh * H:p * CHUNK + (hh + 1) * H],
                                    ps2[:], AF.Silu, bias=t_b2p[:, p:p + 1])

                    # -- L3 X: chambers 0-3, per chunk half -> [128, H]
                    h3 = p_h3.tile([128, 3 * CHUNK // 2], bf16, tag="h3")
                    for hh in range(2):
                        psx = p_mm.tile([128, H], f32, tag="mm")
                        for c in range(4):
                            p = c // 2
                            half = c % 2      # which 64-row half of the pair tile
                            for q in range(H // 512):
                                o = hh * H + q * 512
                                nc.tensor.matmul(
                                    psx[32 * c:32 * (c + 1), q * 512:(q + 1) * 512],
                                    t_w3t[64 * half:64 * half + 64,
                                          c * 32:(c + 1) * 32],
                                    h2[64 * half:64 * half + 64,
                                       p * CHUNK + o:p * CHUNK + o + 512],
                                    tile_position=(64 * half, 32 * c))
                        # (4 chambers run concurrently per q: distinct
                        # (row,col) groups (0,0),(64,32),(0,64),(64,96))
                        nc.scalar.activation(h3[:, hh * H:(hh + 1) * H], psx[:],
                                             AF.Silu, bias=t_b3x[:])

                    # -- L3 Y: c4/c5, chunk halves stacked in partitions:
                    # [0:32]=c4 cols 0:H, [32:64]=c5 cols 0:H,
                    # [64:96]=c4 cols H:2H, [96:128]=c5 cols H:2H
                    psy = p_mm.tile([128, H], f32, tag="mm")
                    for hh in range(2):                 # chunk half
                        for ci, c in enumerate((4, 5)):
                            half = ci                   # c4 lower, c5 upper h2 rows
                            pb = 64 * hh + 32 * ci
                            for q in range(H // 512):
                                o = hh * H + q * 512
                                nc.tensor.matmul(
                                    psy[pb:pb + 32, q * 512:(q + 1) * 512],
                                    t_w3t[64 * half:64 * half + 64,
                                          c * 32:(c + 1) * 32],
                                    h2[64 * half:64 * half + 64,
                                       2 * CHUNK + o:2 * CHUNK + o + 512],
                                    tile_position=(64 * half, pb))
                    nc.scalar.activation(h3[:, CHUNK:CHUNK + H], psy[:],
                                         AF.Silu, bias=t_b3y[:])

                    # -- L4: raw[0:6] accumulating K-stacked matmuls,
                    # one [6, H] strip per chunk half
                    rsb = p_rsb.tile([6, CHUNK], f32, tag="rsb")
                    for hh in range(2):
                        ps4 = p_mm.tile([128, H], f32, tag="mm")
                        for q in range(H // 512):
                            o = hh * H + q * 512
                            nc.tensor.matmul(
                                ps4[0:6, q * 512:(q + 1) * 512], t_w4a[:, 0:6],
                                h3[:, o:o + 512],
                                start=True, stop=False, tile_position=(0, 0))
                            # c4/c5 features: partitions 64*hh:64*hh+64 of h3y
                            nc.tensor.matmul(
                                ps4[0:6, q * 512:(q + 1) * 512],
                                t_w4b[64 * hh:64 * hh + 64, 0:6],
                                h3[64 * hh:64 * hh + 64,
                                   CHUNK + q * 512:CHUNK + (q + 1) * 512],
                                start=False, stop=True,
                                tile_position=(64 * hh, 0))
                        nc.vector.tensor_copy(rsb[:, hh * H:(hh + 1) * H],
                                              ps4[0:6, :])
                    # scatter into batch-major rawbm
                    for c in range(6):
                        nc.gpsimd.dma_start(
                            rawbm[k * PPC:(k + 1) * PPC, c * F:(c + 1) * F],
                            rsb[c:c + 1, :].rearrange("o (a f) -> o a f", f=F))

                # ---- coupling for slice s (batch-major [128, 6F])
                def ctile(tag, w, dt):
                    t = p_cpl.tile([128, w * FMAX], dt, tag=tag)
                    return t[:, 0:w * F]

                # raw outputs don't depend on coupling: emit first so the
                # DMAs overlap the coupling iterations
                r0 = g0 * CHUNK
                for c in range(6):
                    q = nc.sync if c % 2 == 0 else nc.gpsimd
                    q.dma_start(
                        raw_o[c:c + 1, r0:r0 + srows]
                        .rearrange("o (p f) -> (o p) f", f=F),
                        rawbm[:, c * F:(c + 1) * F])

                rawb = ctile("RB", 6, f32)
                nc.vector.tensor_tensor(rawb[:], rawbm[:, 0:6 * F],
                                        t_b4p[:, po:po + 6 * F], OP.add)
                A = ctile("A", 6, f32)
                tt6 = ctile("T6", 6, bf16)
                nc.scalar.activation(tt6[:], rawb[:], AF.Tanh, scale=0.5)
                nc.vector.tensor_scalar(A[:], tt6[:], 0.5, 0.5, OP.mult, OP.add)
                for it in range(CF_ITERS):
                    # D2 = [decay*A | decay*A + pi/2]; one sin ACT covers
                    # sin and cos halves
                    D2 = ctile("D2", 12, f32)
                    D = D2[:, 0:6 * F]
                    nc.vector.tensor_tensor(D, A[:],
                                            t_decp[:, po:po + 6 * F], OP.mult)
                    nc.vector.tensor_scalar(D2[:, 6 * F:12 * F], D,
                                            HALF_PI, None, OP.add)
                    SC = ctile("SC", 12, bf16)
                    nc.scalar.activation(SC[:], D2[:], AF.Sin)
                    PQ = ctile("PQ", 12, bf16)
                    for u in range(2):
                        for (i, j) in init_pairs:
                            nc.vector.tensor_scalar(pq_out(PQ, u, i, j, F),
                                                    pq_src(SC, u, i, j, F),
                                                    float(KC[i][j]), None,
                                                    OP.mult)
                        for (i, j) in rest_pairs:
                            nc.vector.scalar_tensor_tensor(
                                pq_out(PQ, u, i, j, F), pq_src(SC, u, i, j, F),
                                float(KC[i][j]), pq_out(PQ, u, i, j, F),
                                OP.mult, OP.add)
                    U = ctile("U", 12, bf16)
                    nc.vector.tensor_tensor(
                        U.rearrange("p (u x) -> p u x", u=2),
                        PQ.rearrange("p (u x) -> p u x", u=2),
                        swap_u(SC, F), OP.mult)
                    DD = ctile("DD", 6, bf16)
                    nc.vector.tensor_tensor(DD[:], U[:, 0:6 * F],
                                            U[:, 6 * F:12 * F], OP.subtract)
                    V = ctile("V", 6, f32)
                    nc.vector.tensor_tensor(V[:], D, DD[:], OP.add)
                    A = ctile("A", 6, f32)
                    nc.vector.tensor_scalar(A[:], V[:], 0.0, 1.0, OP.max, OP.min)

                # ---- act outputs: contiguous per-chamber DMAs
                # (row = r0 + p*F + f), split across two trigger queues
                for c in range(6):
                    q = nc.sync if c % 2 == 0 else nc.gpsimd
                    q.dma_start(
                        act_o[c:c + 1, r0:r0 + srows]
                        .rearrange("o (p f) -> (o p) f", f=F),
                        A[:, c * F:(c + 1) * F])
                g0 += chunks_per_slice
    return nc, emit


def prep_weights(W1, b1, W2, b2, W3, b3, W4, b4, slice_sizes):
    """Host-side weight layout preparation."""
    import ml_dtypes
    bf16 = ml_dtypes.bfloat16
    d = {}
    d["w1t"] = np.ascontiguousarray(
        W1.transpose(2, 0, 1).reshape(RES_DIM, 6 * 128)).astype(bf16)
    d["b1t"] = np.ascontiguousarray(b1.T)                      # [128, 6]
    d["w2t"] = np.ascontiguousarray(
        W2.transpose(2, 0, 1).reshape(128, 6 * 64)).astype(bf16)
    b2p = np.zeros((128, 3), np.float32)
    for p in range(3):
        b2p[0:64, p] = b2[2 * p]
        b2p[64:128, p] = b2[2 * p + 1]
    d["b2p"] = b2p
    w3t_h = W3.transpose(2, 0, 1).reshape(64, 6 * 32)
    d["w3t"] = np.ascontiguousarray(
        np.concatenate([w3t_h, w3t_h], axis=0)).astype(bf16)
    b3x = np.zeros((128, 1), np.float32)
    for c in range(4):
        b3x[32 * c:32 * (c + 1), 0] = b3[c]
    d["b3x"] = b3x
    b3y = np.zeros((128, 1), np.float32)
    b3y[0:32, 0] = b3[4]
    b3y[32:64, 0] = b3[5]
    b3y[64:96, 0] = b3[4]
    b3y[96:128, 0] = b3[5]
    d["b3y"] = b3y
    w4a = np.zeros((128, 6), np.float32)
    for c in range(4):
        w4a[32 * c:32 * (c + 1), c] = W4[c, 0, :]
    d["w4a"] = w4a.astype(bf16)
    w4b = np.zeros((128, 6), np.float32)
    w4b[0:32, 4] = W4[4, 0, :]
    w4b[32:64, 5] = W4[5, 0, :]
    w4b[64:96, 4] = W4[4, 0, :]
    w4b[96:128, 5] = W4[5, 0, :]
    d["w4b"] = w4b.astype(bf16)
    d["cst"] = np.full((128, 1), HALF_PI, np.float32)
    fs = sorted({s // 128 for s in slice_sizes}, reverse=True)
    patw = sum(6 * F for F in fs)
    decp = np.zeros((128, patw), np.float32)
    b4p = np.zeros((128, patw), np.float32)
    off = 0
    for F in fs:
        for c in range(6):
            decp[:, off + c * F:off + (c + 1) * F] = DECAY[c]
            b4p[:, off + c * F:off + (c + 1) * F] = b4[c, 0]
        off += 6 * F
    d["decp"] = decp
    d["b4p"] = b4p
    d["_b4"] = np.ascontiguousarray(b4[:, 0])                  # host-only
    return d


def build_program(R=R_CORE, slice_sizes=DEFAULT_SLICES):
    """Build + bacc-compile the program (cached)."""
    key = (R, tuple(slice_sizes))
    if key in _BUILD_CACHE:
        return _BUILD_CACHE[key]
    nc, emit = _build(R, list(slice_sizes))
    emit()
    nc.compile()
    _BUILD_CACHE[key] = nc
    return nc


def kernel(res, W1, b1, W2, b2, W3, b3, W4, b4, coupling):
    """Full-input entry point: shards res over 8 cores, runs the SPMD
    kernel, gathers and returns (act, raw) like the reference."""
    from concourse.bass_utils import run_bass_kernel_spmd

    res = np.ascontiguousarray(np.asarray(res, np.float32))
    W1 = np.asarray(W1, np.float32); b1 = np.asarray(b1, np.float32)
    W2 = np.asarray(W2, np.float32); b2 = np.asarray(b2, np.float32)
    W3 = np.asarray(W3, np.float32); b3 = np.asarray(b3, np.float32)
    W4 = np.asarray(W4, np.float32); b4 = np.asarray(b4, np.float32)

    wd = prep_weights(W1, b1, W2, b2, W3, b3, W4, b4, DEFAULT_SLICES)
    b4vec = wd.pop("_b4")
    nc = build_program(R_CORE)

    import ml_dtypes
    res_t = np.ascontiguousarray(res.T.astype(ml_dtypes.bfloat16))  # [100, B]
    in_maps = []
    for i in range(N_CORES):
        m = dict(wd)
        m["res_t"] = np.ascontiguousarray(res_t[:, i * R_CORE:(i + 1) * R_CORE])
        in_maps.append(m)
    out = run_bass_kernel_spmd(nc, in_maps, list(range(N_CORES)))
    act = np.concatenate(
        [np.ascontiguousarray(out.results[i]["act_o"].T) for i in range(N_CORES)],
        axis=0)
    raw = np.concatenate(
        [np.ascontiguousarray(out.results[i]["raw_o"].T) for i in range(N_CORES)],
        axis=0)
    raw = raw + b4vec[None, :]
    return act.astype(np.float32), raw.astype(np.float32)


# revision 25
# speedup vs baseline: 1.0265x; 1.0063x over previous
"""Trainium2 Bass kernel for nn_Chambers: 6 per-chamber MLPs over a shared
reservoir input, followed by 5 coupled-chamber fixed-point iterations.

Data-parallel over 8 NeuronCores: each core processes B/8 = 32768 rows.

v2 design (ACT-engine-bound; see work/ notes):
  - MLP is feature-major with [128,2048] PSUM tiles (2-tile rotation = all
    8 banks) so every silu ACT op is as large as PSUM allows:
    L1 = 6 ops/chunk (per-chamber, per-partition bias), L2 = 3 ops/chunk
    (pair-packed), L3 = 2 ops/chunk (c0-3 full tile + c4/c5 stacked across
    chunk halves so no ACT lanes are wasted).
  - L4 per-chamber dot products are K-stacked accumulating matmuls into a
    [6,2048] PSUM strip that is DMA-scattered directly into the batch-major
    rawbm tile (no [6,*] DVE copies anywhere).
  - Coupling runs batch-major [128, 6F] with sin|cos concatenated in one
    [128,12F] bf16 tile; the 15 symmetric C-pair AXPYs update P and Q
    together through 4D strided views (bf16 -> DVE 2x), decay is one
    pattern-tile multiply, and sigmoid is one merged tanh (b4 folded via a
    pattern add). State stays f32.
  - Outputs are written per chamber as contiguous [128,F] DMAs into
    chamber-major [6,R] HBM tensors; host transposes (row = r0 + p*F + f
    is exactly the slice row order).
"""

import numpy as np

# ---- problem constants (fixed by the task; kernel.py must be self-contained)
B = 262144
RES_DIM = 100
NCH = 6
CF_ITERS = 5
CF_K = 0.02
DECAY = np.array([0.9, 0.93, 0.85, 0.97, 0.88, 0.94], dtype=np.float32)
COUPLING = np.array([
    [0.0, -0.3, 0.6, 0.4, -0.2, 0.3],
    [-0.3, 0.0, -0.5, -0.7, 0.6, 0.4],
    [0.6, -0.5, 0.0, 0.3, -0.3, 0.2],
    [0.4, -0.7, 0.3, 0.0, -0.4, 0.5],
    [-0.2, 0.6, -0.3, -0.4, 0.0, 0.3],
    [0.3, 0.4, 0.2, 0.5, 0.3, 0.0]], dtype=np.float32)
N_CORES = 8
R_CORE = B // N_CORES          # 32768 rows per core
CHUNK = 2048                   # rows per MLP chunk
HALF_PI = float(np.pi / 2.0)
DEFAULT_SLICES = (8192, 8192, 8192, 4096, 4096)

_BUILD_CACHE = {}


def _build(R, slice_sizes):
    """Emit + compile the per-core SPMD program."""
    from contextlib import ExitStack
    import concourse.bass as bass
    import concourse.mybir as mybir
    from concourse import bacc, tile

    f32 = mybir.dt.float32
    bf16 = mybir.dt.bfloat16
    AF = mybir.ActivationFunctionType
    OP = mybir.AluOpType

    assert sum(slice_sizes) == R and all(s % CHUNK == 0 for s in slice_sizes)
    KC = (CF_K * COUPLING).astype(np.float64)
    init_pairs = [(0, 1), (2, 3), (4, 5)]
    rest_pairs = [(i, j) for i in range(6) for j in range(i + 1, 6)
                  if (i, j) not in init_pairs]
    fs = sorted({s // 128 for s in slice_sizes}, reverse=True)  # distinct F values
    f_off = {}
    off = 0
    for F in fs:
        f_off[F] = off
        off += 6 * F
    PATW = off                                   # pattern tensor width

    nc = bacc.Bacc("TRN2", target_bir_lowering=False, debug=False,
                   num_devices=N_CORES)
    res = nc.dram_tensor("res_t", [RES_DIM, R], bf16, kind="ExternalInput").ap()
    w1t = nc.dram_tensor("w1t", [RES_DIM, 6 * 128], bf16, kind="ExternalInput").ap()
    b1t = nc.dram_tensor("b1t", [128, 6], f32, kind="ExternalInput").ap()
    w2t = nc.dram_tensor("w2t", [128, 6 * 64], bf16, kind="ExternalInput").ap()
    b2p = nc.dram_tensor("b2p", [128, 3], f32, kind="ExternalInput").ap()
    # w3t holds W3^T twice (rows 0-63 and 64-127) for row-tiled matmuls
    w3t = nc.dram_tensor("w3t", [128, 6 * 32], bf16, kind="ExternalInput").ap()
    b3x = nc.dram_tensor("b3x", [128, 1], f32, kind="ExternalInput").ap()
    b3y = nc.dram_tensor("b3y", [128, 1], f32, kind="ExternalInput").ap()
    w4a = nc.dram_tensor("w4a", [128, 6], bf16, kind="ExternalInput").ap()
    w4b = nc.dram_tensor("w4b", [128, 6], bf16, kind="ExternalInput").ap()
    cst = nc.dram_tensor("cst", [128, 1], f32, kind="ExternalInput").ap()
    decp = nc.dram_tensor("decp", [128, PATW], f32, kind="ExternalInput").ap()
    b4p = nc.dram_tensor("b4p", [128, PATW], f32, kind="ExternalInput").ap()
    act_o = nc.dram_tensor("act_o", [6, R], f32, kind="ExternalOutput").ap()
    raw_o = nc.dram_tensor("raw_o", [6, R], f32, kind="ExternalOutput").ap()

    def emit():
        with tile.TileContext(nc) as tc, ExitStack() as ctx:
            wp = ctx.enter_context(tc.tile_pool(name="w", bufs=1))
            # critical-path weights (L1 needs these first) on the sync
            # HWDGE queue; the rest on gpsimd; coupling patterns (first
            # used ~100us in) last
            t_w1t = wp.tile([RES_DIM, 6 * 128], bf16, tag="w1t")
            nc.sync.dma_start(t_w1t[:], w1t)
            t_b1t = wp.tile([128, 6], f32, tag="b1t")
            nc.sync.dma_start(t_b1t[:], b1t)
            t_w2t = wp.tile([128, 6 * 64], bf16, tag="w2t")
            nc.gpsimd.dma_start(t_w2t[:], w2t)
            t_b2p = wp.tile([128, 3], f32, tag="b2p")
            nc.gpsimd.dma_start(t_b2p[:], b2p)
            t_w3t = wp.tile([128, 6 * 32], bf16, tag="w3t")
            nc.gpsimd.dma_start(t_w3t[:], w3t)
            t_b3x = wp.tile([128, 1], f32, tag="b3x")
            nc.gpsimd.dma_start(t_b3x[:], b3x)
            t_b3y = wp.tile([128, 1], f32, tag="b3y")
            nc.gpsimd.dma_start(t_b3y[:], b3y)
            t_w4a = wp.tile([128, 6], bf16, tag="w4a")
            nc.gpsimd.dma_start(t_w4a[:], w4a)
            t_w4b = wp.tile([128, 6], bf16, tag="w4b")
            nc.gpsimd.dma_start(t_w4b[:], w4b)
            t_cst = wp.tile([128, 1], f32, tag="cst")
            nc.gpsimd.dma_start(t_cst[:], cst)
            t_decp = wp.tile([128, PATW], f32, tag="decp")
            nc.gpsimd.dma_start(t_decp[:], decp)
            t_b4p = wp.tile([128, PATW], f32, tag="b4p")
            nc.gpsimd.dma_start(t_b4p[:], b4p)

            p_rT = ctx.enter_context(tc.tile_pool(name="rT", bufs=3))
            p_mm = ctx.enter_context(tc.tile_pool(name="pmm", bufs=4, space="PSUM"))
            p_h1 = ctx.enter_context(tc.tile_pool(name="h1", bufs=2))
            p_h2 = ctx.enter_context(tc.tile_pool(name="h2", bufs=2))
            p_h3 = ctx.enter_context(tc.tile_pool(name="h3", bufs=2))
            p_rsb = ctx.enter_context(tc.tile_pool(name="rsb", bufs=2))
            p_bm = ctx.enter_context(tc.tile_pool(name="bm", bufs=2))
            p_cpl = ctx.enter_context(tc.tile_pool(name="cpl", bufs=2))

            # 3D pair views on one u-half of a [128, 12F] tile (u=0 first
            # 6F columns, u=1 second): verifier caps TS/STT APs at 3 dims.
            def pq_out(t_ap, u, i, j, F):
                x = t_ap[:, u * 6 * F:(u + 1) * 6 * F].rearrange(
                    "p (c f) -> p c f", f=F)
                return x[:, i:j + 1:(j - i), :]

            def pq_src(t_ap, u, i, j, F):
                # c-blocks [j, i] (swapped) via negative step
                x = t_ap[:, u * 6 * F:(u + 1) * 6 * F].rearrange(
                    "p (c f) -> p c f", f=F)
                if i == 0:
                    return x[:, j::-(j - i), :][:, 0:2, :]
                return x[:, j:i - 1:-(j - i), :]

            def swap_u(t_ap, F):
                x = t_ap.rearrange("p (u x) -> p u x", u=2)
                return x[:, 1::-1, :]

            FMAX = max(slice_sizes) // 128
            g0 = 0
            for s, srows in enumerate(slice_sizes):
                F = srows // 128
                PPC = CHUNK // F
                po = f_off[F]
                chunks_per_slice = srows // CHUNK
                rawbm = p_bm.tile([128, 6 * FMAX], f32, tag="rawbm")
                for k in range(chunks_per_slice):
                    g = g0 + k                         # global chunk id
                    rT = p_rT.tile([RES_DIM, CHUNK], bf16, tag="rT")
                    nc.sync.dma_start(rT[:], res[:, g * CHUNK:(g + 1) * CHUNK])

                    # -- L1 (+ L2 when a pair completes); [128,1024] PSUM
                    # tiles, 4-deep rotation
                    h1 = p_h1.tile([128, 6 * CHUNK], bf16, tag="h1")
                    h2 = p_h2.tile([128, 3 * CHUNK], bf16, tag="h2")
                    H = CHUNK // 2
                    for c in range(6):
                        for hh in range(2):
                            ps = p_mm.tile([128, H], f32, tag="mm")
                            for q in range(H // 512):
                                o = hh * H + q * 512
                                nc.tensor.matmul(
                                    ps[:, q * 512:(q + 1) * 512],
                                    t_w1t[:, c * 128:(c + 1) * 128],
                                    rT[:, o:o + 512])
                            nc.scalar.activation(
                                h1[:, c * CHUNK + hh * H:c * CHUNK + (hh + 1) * H],
                                ps[:], AF.Silu, bias=t_b1t[:, c:c + 1])
                        if c % 2 == 1:
                            p = c // 2
                            for hh in range(2):
                                ps2 = p_mm.tile([128, H], f32, tag="mm")
                                for q in range(H // 512):
                                    o = hh * H + q * 512
                                    nc.tensor.matmul(
                                        ps2[0:64, q * 512:(q + 1) * 512],
                                        t_w2t[:, (2 * p) * 64:(2 * p + 1) * 64],
                                        h1[:, 2 * p * CHUNK + o:
                                           2 * p * CHUNK + o + 512],
                                        tile_position=(0, 0))
                                    nc.tensor.matmul(
                                        ps2[64:128, q * 512:(q + 1) * 512],
                                        t_w2t[:, (2 * p + 1) * 64:(2 * p + 2) * 64],
                                        h1[:, (2 * p + 1) * CHUNK + o:
                                           (2 * p + 1) * CHUNK + o + 512],
                                        tile_position=(0, 64))
                                nc.scalar.activation(
                                    h2[:, p * CHUNK + hh * H:p * CHUNK + (hh + 1) * H],
                                    ps2[:], AF.Silu, bias=t_b2p[:, p:p + 1])

                    # -- L3 X: chambers 0-3, per chunk half -> [128, H]
                    h3 = p_h3.tile([128, 3 * CHUNK // 2], bf16, tag="h3")
                    for hh in range(2):
                        psx = p_mm.tile([128, H], f32, tag="mm")
                        for c in range(4):
                            p = c // 2
                            half = c % 2      # which 64-row half of the pair tile
                            for q in range(H // 512):
                                o = hh * H + q * 512
                                nc.tensor.matmul(
                                    psx[32 * c:32 * (c + 1), q * 512:(q + 1) * 512],
                                    t_w3t[64 * half:64 * half + 64,
                                          c * 32:(c + 1) * 32],
                                    h2[64 * half:64 * half + 64,
                                       p * CHUNK + o:p * CHUNK + o + 512],
                                    tile_position=(64 * half, 32 * c))
                        # (4 chambers run concurrently per q: distinct
                        # (row,col) groups (0,0),(64,32),(0,64),(64,96))
                        nc.scalar.activation(h3[:, hh * H:(hh + 1) * H], psx[:],
                                             AF.Silu, bias=t_b3x[:])

                    # -- L3 Y: c4/c5, chunk halves stacked in partitions:
                    # [0:32]=c4 cols 0:H, [32:64]=c5 cols 0:H,
                    # [64:96]=c4 cols H:2H, [96:128]=c5 cols H:2H
                    psy = p_mm.tile([128, H], f32, tag="mm")
                    for hh in range(2):                 # chunk half
                        for ci, c in enumerate((4, 5)):
                            half = ci                   # c4 lower, c5 upper h2 rows
                            pb = 64 * hh + 32 * ci
                            for q in range(H // 512):
                                o = hh * H + q * 512
                                nc.tensor.matmul(
                                    psy[pb:pb + 32, q * 512:(q + 1) * 512],
                                    t_w3t[64 * half:64 * half + 64,
                                          c * 32:(c + 1) * 32],
                                    h2[64 * half:64 * half + 64,
                                       2 * CHUNK + o:2 * CHUNK + o + 512],
                                    tile_position=(64 * half, pb))
                    nc.scalar.activation(h3[:, CHUNK:CHUNK + H], psy[:],
                                         AF.Silu, bias=t_b3y[:])

                    # -- L4: raw[0:6] accumulating K-stacked matmuls,
                    # one [6, H] strip per chunk half
                    rsb = p_rsb.tile([6, CHUNK], f32, tag="rsb")
                    for hh in range(2):
                        ps4 = p_mm.tile([128, H], f32, tag="mm")
                        for q in range(H // 512):
                            o = hh * H + q * 512
                            nc.tensor.matmul(
                                ps4[0:6, q * 512:(q + 1) * 512], t_w4a[:, 0:6],
                                h3[:, o:o + 512],
                                start=True, stop=False, tile_position=(0, 0))
                            # c4/c5 features: partitions 64*hh:64*hh+64 of h3y
                            nc.tensor.matmul(
                                ps4[0:6, q * 512:(q + 1) * 512],
                                t_w4b[64 * hh:64 * hh + 64, 0:6],
                                h3[64 * hh:64 * hh + 64,
                                   CHUNK + q * 512:CHUNK + (q + 1) * 512],
                                start=False, stop=True,
                                tile_position=(64 * hh, 0))
                        nc.vector.tensor_copy(rsb[:, hh * H:(hh + 1) * H],
                                              ps4[0:6, :])
                    # scatter into batch-major rawbm
                    for c in range(6):
                        nc.gpsimd.dma_start(
                            rawbm[k * PPC:(k + 1) * PPC, c * F:(c + 1) * F],
                            rsb[c:c + 1, :].rearrange("o (a f) -> o a f", f=F))

                # ---- coupling for slice s (batch-major [128, 6F])
                def ctile(tag, w, dt):
                    t = p_cpl.tile([128, w * FMAX], dt, tag=tag)
                    return t[:, 0:w * F]

                # raw outputs don't depend on coupling: emit first so the
                # DMAs overlap the coupling iterations
                r0 = g0 * CHUNK
                for c in range(6):
                    q = nc.sync if c % 2 == 0 else nc.gpsimd
                    q.dma_start(
                        raw_o[c:c + 1, r0:r0 + srows]
                        .rearrange("o (p f) -> (o p) f", f=F),
                        rawbm[:, c * F:(c + 1) * F])

                rawb = ctile("RB", 6, f32)
                nc.vector.tensor_tensor(rawb[:], rawbm[:, 0:6 * F],
                                        t_b4p[:, po:po + 6 * F], OP.add)
                A = ctile("A", 6, f32)
                tt6 = ctile("T6", 6, bf16)
                nc.scalar.activation(tt6[:], rawb[:], AF.Tanh, scale=0.5)
                nc.vector.tensor_scalar(A[:], tt6[:], 0.5, 0.5, OP.mult, OP.add)
                for it in range(CF_ITERS):
                    # D2 = [decay*A | decay*A + pi/2]; one sin ACT covers
                    # sin and cos halves
                    D2 = ctile("D2", 12, f32)
                    D = D2[:, 0:6 * F]
                    nc.vector.tensor_tensor(D, A[:],
                                            t_decp[:, po:po + 6 * F], OP.mult)
                    nc.vector.tensor_scalar(D2[:, 6 * F:12 * F], D,
                                            HALF_PI, None, OP.add)
                    SC = ctile("SC", 12, bf16)
                    nc.scalar.activation(SC[:], D2[:], AF.Sin)
                    PQ = ctile("PQ", 12, bf16)
                    for u in range(2):
                        for (i, j) in init_pairs:
                            nc.vector.tensor_scalar(pq_out(PQ, u, i, j, F),
                                                    pq_src(SC, u, i, j, F),
                                                    float(KC[i][j]), None,
                                                    OP.mult)
                        for (i, j) in rest_pairs:
                            nc.vector.scalar_tensor_tensor(
                                pq_out(PQ, u, i, j, F), pq_src(SC, u, i, j, F),
                                float(KC[i][j]), pq_out(PQ, u, i, j, F),
                                OP.mult, OP.add)
                    U = ctile("U", 12, bf16)
                    nc.vector.tensor_tensor(
                        U.rearrange("p (u x) -> p u x", u=2),
                        PQ.rearrange("p (u x) -> p u x", u=2),
                        swap_u(SC, F), OP.mult)
                    DD = ctile("DD", 6, bf16)
                    nc.vector.tensor_tensor(DD[:], U[:, 0:6 * F],
                                            U[:, 6 * F:12 * F], OP.subtract)
                    V = ctile("V", 6, f32)
                    nc.vector.tensor_tensor(V[:], D, DD[:], OP.add)
                    A = ctile("A", 6, f32)
                    nc.vector.tensor_scalar(A[:], V[:], 0.0, 1.0, OP.max, OP.min)

                # ---- act outputs: contiguous per-chamber DMAs
                # (row = r0 + p*F + f), split across two trigger queues
                for c in range(6):
                    q = nc.sync if c % 2 == 0 else nc.gpsimd
                    q.dma_start(
                        act_o[c:c + 1, r0:r0 + srows]
                        .rearrange("o (p f) -> (o p) f", f=F),
                        A[:, c * F:(c + 1) * F])
                g0 += chunks_per_slice
    return nc, emit


def prep_weights(W1, b1, W2, b2, W3, b3, W4, b4, slice_sizes):
    """Host-side weight layout preparation."""
    import ml_dtypes
    bf16 = ml_dtypes.bfloat16
    d = {}
    d["w1t"] = np.ascontiguousarray(
        W1.transpose(2, 0, 1).reshape(RES_DIM, 6 * 128)).astype(bf16)
    d["b1t"] = np.ascontiguousarray(b1.T)                      # [128, 6]
    d["w2t"] = np.ascontiguousarray(
        W2.transpose(2, 0, 1).reshape(128, 6 * 64)).astype(bf16)
    b2p = np.zeros((128, 3), np.float32)
    for p in range(3):
        b2p[0:64, p] = b2[2 * p]
        b2p[64:128, p] = b2[2 * p + 1]
    d["b2p"] = b2p
    w3t_h = W3.transpose(2, 0, 1).reshape(64, 6 * 32)
    d["w3t"] = np.ascontiguousarray(
        np.concatenate([w3t_h, w3t_h], axis=0)).astype(bf16)
    b3x = np.zeros((128, 1), np.float32)
    for c in range(4):
        b3x[32 * c:32 * (c + 1), 0] = b3[c]
    d["b3x"] = b3x
    b3y = np.zeros((128, 1), np.float32)
    b3y[0:32, 0] = b3[4]
    b3y[32:64, 0] = b3[5]
    b3y[64:96, 0] = b3[4]
    b3y[96:128, 0] = b3[5]
    d["b3y"] = b3y
    w4a = np.zeros((128, 6), np.float32)
    for c in range(4):
        w4a[32 * c:32 * (c + 1), c] = W4[c, 0, :]
    d["w4a"] = w4a.astype(bf16)
    w4b = np.zeros((128, 6), np.float32)
    w4b[0:32, 4] = W4[4, 0, :]
    w4b[32:64, 5] = W4[5, 0, :]
    w4b[64:96, 4] = W4[4, 0, :]
    w4b[96:128, 5] = W4[5, 0, :]
    d["w4b"] = w4b.astype(bf16)
    d["cst"] = np.full((128, 1), HALF_PI, np.float32)
    fs = sorted({s // 128 for s in slice_sizes}, reverse=True)
    patw = sum(6 * F for F in fs)
    decp = np.zeros((128, patw), np.float32)
    b4p = np.zeros((128, patw), np.float32)
    off = 0
    for F in fs:
        for c in range(6):
            decp[:, off + c * F:off + (c + 1) * F] = DECAY[c]
            b4p[:, off + c * F:off + (c + 1) * F] = b4[c, 0]
        off += 6 * F
    d["decp"] = decp
    d["b4p"] = b4p
    d["_b4"] = np.ascontiguousarray(b4[:, 0])                  # host-only
    return d


def build_program(R=R_CORE, slice_sizes=DEFAULT_SLICES):
    """Build + bacc-compile the program (cached)."""
    key = (R, tuple(slice_sizes))
    if key in _BUILD_CACHE:
        return _BUILD_CACHE[key]
    nc, emit = _build(R, list(slice_sizes))
    emit()
    nc.compile()
    _BUILD_CACHE[key] = nc
    return nc


def kernel(res, W1, b1, W2, b2, W3, b3, W4, b4, coupling):
    """Full-input entry point: shards res over 8 cores, runs the SPMD
    kernel, gathers and returns (act, raw) like the reference."""
    from concourse.bass_utils import run_bass_kernel_spmd

    res = np.ascontiguousarray(np.asarray(res, np.float32))
    W1 = np.asarray(W1, np.float32); b1 = np.asarray(b1, np.float32)
    W2 = np.asarray(W2, np.float32); b2 = np.asarray(b2, np.float32)
    W3 = np.asarray(W3, np.float32); b3 = np.asarray(b3, np.float32)
    W4 = np.asarray(W4, np.float32); b4 = np.asarray(b4, np.float32)

    wd = prep_weights(W1, b1, W2, b2, W3, b3, W4, b4, DEFAULT_SLICES)
    b4vec = wd.pop("_b4")
    nc = build_program(R_CORE)

    import ml_dtypes
    res_t = np.ascontiguousarray(res.T.astype(ml_dtypes.bfloat16))  # [100, B]
    in_maps = []
    for i in range(N_CORES):
        m = dict(wd)
        m["res_t"] = np.ascontiguousarray(res_t[:, i * R_CORE:(i + 1) * R_CORE])
        in_maps.append(m)
    out = run_bass_kernel_spmd(nc, in_maps, list(range(N_CORES)))
    act = np.concatenate(
        [np.ascontiguousarray(out.results[i]["act_o"].T) for i in range(N_CORES)],
        axis=0)
    raw = np.concatenate(
        [np.ascontiguousarray(out.results[i]["raw_o"].T) for i in range(N_CORES)],
        axis=0)
    raw = raw + b4vec[None, :]
    return act.astype(np.float32), raw.astype(np.float32)
